# revision 1
# baseline (speedup 1.0000x reference)
# MultiLoraConv2d kernel for 8 trn2 NeuronCores (Bass/Tile, data-parallel over batch).
#
# Math (per sample b):
#   delta_flat[b] = sum_t 2*alphas[b,t] * (lora_B[t] @ lora_A[t])        [768, 768]
#   agg[b] = W + delta_flat[b].reshape(COUT, CIN, 3, 3)                  (flat reinterpret)
#   out[b] = conv2d(x[b], agg[b], pad=1)
#
# Device strategy (per core, S = B/8 samples):
#   - Host pre-lays-out tensors partition-first; LoRA factors regrouped by
#     (d = 3*i + j, s = (c*9+d)//768) so per-sample aggregated conv weights
#     come out of the PE directly in c-major (stationary) layout:
#       S_d[c, o] = sum_s sum_r a3[d,s,r,c] * (2*alpha_{t(r)} * b3[s,r,o])
#   - Conv = 18 shifted matmuls (9 taps x 2 cin tiles) accumulated per PSUM
#     bank, x zero-padded (66x66) in SBUF, all matmuls fp32r (full-rate).
import numpy as np

B, T, R, ALPHA = 32, 4, 8, 16
CIN, COUT, K = 256, 256, 3
H = W_SP = 64
SCALING = ALPHA / R
NCORES = 8
S = B // NCORES      # samples per core
NR = T * R * K       # 96 lora rows (padded to 128 partitions)
P = 128
HP = H + 2           # 66 padded
HH = 34              # padded-row half-tile height (rows 0:34 and 32:66)

_CACHE = {}


def _build_nc():
    import concourse.bacc as bacc
    import concourse.mybir as mybir
    import concourse.tile as tile

    f32 = mybir.dt.float32
    f32r = mybir.dt.float32r

    nc = bacc.Bacc("TRN2", target_bir_lowering=False, debug=False, num_devices=NCORES)

    xp = nc.declare_dram_parameter("xp", [S, 2, P, HP, HP], f32r, isOutput=False)
    wt = nc.declare_dram_parameter("wt", [P, 9, 2, COUT], f32, isOutput=False)
    a3 = nc.declare_dram_parameter("a3", [P, 9, 3, CIN], f32r, isOutput=False)
    b3 = nc.declare_dram_parameter("b3", [P, 3, COUT], f32, isOutput=False)
    alph = nc.declare_dram_parameter("alph", [P, S], f32, isOutput=False)
    outd = nc.declare_dram_parameter("out", [S, 2, P, H, W_SP], f32, isOutput=True)

    HSMP = S // 2  # samples per 512-wide delta-matmul half

    with tile.TileContext(nc) as tc:
        with tc.tile_pool(name="ws_pool", bufs=2 * S) as ws_pool, \
             tc.tile_pool(name="xt_pool", bufs=8) as xt_pool:
            # per-(sample, cin-tile) aggregated conv weights, c-major
            ws = [ws_pool.tile([P, 9, COUT], f32r, name="ws") for _ in range(2 * S)]

            # ---- phase 1: aggregated weights via LoRA matmuls ----
            with tc.tile_pool(name="cst", bufs=1) as cst, \
                 tc.tile_pool(name="dps", bufs=4, space="PSUM") as dps:
                a3_sb = cst.tile([P, 9, 3, CIN], f32r)
                b3_sb = cst.tile([P, 3, COUT], f32)
                alph_sb = cst.tile([P, S], f32)
                wt_sb = cst.tile([P, 9, 2, COUT], f32)
                b3s_lo = cst.tile([P, 3, S // 2, COUT], f32r)
                b3s_hi = cst.tile([P, 3, S // 2, COUT], f32r)

                # DMA transfers complete strictly in issue order at ~290GB/s
                # (each dma_start fans over all 16 engines). Issue in the
                # order the pipeline consumes: small consts, a3 chunks (gate
                # the delta matmuls), wt chunks (gate evictions; dps bufs=4
                # gives ~11us of eviction slack), then sample-0 x.
                nc.sync.dma_start(b3_sb[:, :, :], b3[:, :, :])
                nc.sync.dma_start(alph_sb[:, :], alph[:, :])
                nc.sync.dma_start(a3_sb[:, 0:1], a3[:, 0:1])
                nc.sync.dma_start(a3_sb[:, 1:3], a3[:, 1:3])
                nc.sync.dma_start(wt_sb[:, 0:1], wt[:, 0:1])
                nc.sync.dma_start(a3_sb[:, 3:6], a3[:, 3:6])
                nc.sync.dma_start(wt_sb[:, 1:3], wt[:, 1:3])
                nc.sync.dma_start(a3_sb[:, 6:9], a3[:, 6:9])
                nc.sync.dma_start(wt_sb[:, 3:9], wt[:, 3:9])
                # sample-0 x as (ct, h-half) quarters, lower halves first so
                # the first conv bank-groups can start before the rest lands
                xt0 = [[None, None], [None, None]]
                for h in range(2):
                    for ct in range(2):
                        t = xt_pool.tile([P, HH, HP], f32r, name="xt")
                        nc.sync.dma_start(t[:, :, :], xp[0, ct, :, 32 * h:32 * h + HH, :])
                        xt0[ct][h] = t

                nc.vector.tensor_scalar_mul(alph_sb[:, :], alph_sb[:, :],
                                            float(SCALING))
                for smp in range(S):
                    dst = b3s_lo if smp < HSMP else b3s_hi
                    for s in range(3):
                        nc.vector.tensor_scalar_mul(
                            dst[:, s, smp % HSMP, :], b3_sb[:, s, :],
                            alph_sb[:, smp:smp + 1])

                for d in range(9):
                    for ct in range(2):
                        dp = dps.tile([P, 2, HSMP, COUT], f32, name="dp")
                        for half in range(2):
                            for s in range(3):
                                nc.tensor.matmul(
                                    dp[:, half, :, :],
                                    a3_sb[:, d, s, ct * P:(ct + 1) * P],
                                    (b3s_lo if half == 0 else b3s_hi)[:, s, :, :],
                                    start=(s == 0), stop=(s == 2))
                        for smp in range(S):
                            nc.vector.tensor_add(
                                ws[smp * 2 + ct][:, d, :],
                                dp[:, smp // HSMP, smp % HSMP, :],
                                wt_sb[:, d, ct, :])

            # ---- phase 2: per-sample conv, 18 shifted matmuls per psum bank ----
            with tc.tile_pool(name="ob_pool", bufs=4) as ob_pool, \
                 tc.tile_pool(name="cps", bufs=8, space="PSUM") as cps:
                for smp in range(S):
                    if smp == 0:
                        xts = xt0
                    else:
                        xts = [[None, None], [None, None]]
                        for h in range(2):
                            for ct in range(2):
                                t = xt_pool.tile([P, HH, HP], f32r, name="xt")
                                nc.sync.dma_start(
                                    t[:, :, :], xp[smp, ct, :, 32 * h:32 * h + HH, :])
                                xts[ct][h] = t
                    for ot in range(2):
                        for hb in range(8):
                            pb = cps.tile([P, 8, W_SP], f32, name="pb")
                            first = True
                            for ct in range(2):
                                for d in range(9):
                                    di, dj = divmod(d, 3)
                                    loc = (hb % 4) * 8 + di
                                    nc.tensor.matmul(
                                        pb[:, :, :],
                                        ws[smp * 2 + ct][:, d, ot * P:(ot + 1) * P],
                                        xts[ct][hb // 4][:, loc:loc + 8,
                                                         dj:dj + W_SP],
                                        start=first, stop=(ct == 1 and d == 8))
                                    first = False
                            ob = ob_pool.tile([P, 8, W_SP], f32, name="ob")
                            nc.vector.tensor_copy(ob[:, :, :], pb[:, :, :])
                            nc.sync.dma_start(
                                outd[smp, ot, :, hb * 8:(hb + 1) * 8, :],
                                ob[:, :, :])
    nc.finalize()
    return nc


def _host_prep(x, alphas, W, lora_A, lora_B):
    """Host-side layout-only transforms (pad/transpose/gather/replicate)."""
    xf = np.ascontiguousarray(np.asarray(x, dtype=np.float32))
    af = np.asarray(alphas, dtype=np.float32)
    Wf = np.asarray(W, dtype=np.float32)
    Af = np.asarray(lora_A, dtype=np.float32).reshape(NR, CIN * K)   # Acat
    Bf = np.asarray(lora_B, dtype=np.float32)

    # padded x, per core: (S, 2, 128, 66, 66)
    xpad = np.zeros((B, CIN, HP, HP), np.float32)
    xpad[:, :, 1:-1, 1:-1] = xf
    xpad = xpad.reshape(NCORES, S, 2, P, HP, HP)

    # base weights c-major, d-major free layout: wt[p, d, ct, o]
    wth = np.ascontiguousarray(
        Wf.reshape(COUT, CIN, 9).transpose(1, 2, 0)        # [c, d, o]
        .reshape(2, P, 9, COUT)                            # [ct, p, d, o]
        .transpose(1, 2, 0, 3))                            # [p, d, ct, o]

    # a3[r, d, s, c] = Acat[r, c*9+d-768*s] masked; rows padded 96 -> 128
    a3h = np.zeros((P, 9, 3, CIN), np.float32)
    cc = np.arange(CIN)
    for d in range(9):
        q = cc * 9 + d
        s_of_c = q // (CIN * K)
        q_of_c = q % (CIN * K)
        for s in range(3):
            m = s_of_c == s
            a3h[:NR, d, s, m] = Af[:, q_of_c[m]]

    # b3[r, s, o] = Bcat[3o+s, r];  Bcat = lora_B transposed to [768, 96]
    Bcat = Bf.transpose(1, 0, 2).reshape(COUT * K, NR)
    b3h = np.zeros((P, 3, COUT), np.float32)
    b3h[:NR] = Bcat.reshape(COUT, 3, NR).transpose(2, 1, 0)

    # alph[r, smp] per core (repeat each task 24x; zero rows >= 96)
    alphh = np.zeros((NCORES, P, S), np.float32)
    rep = np.repeat(af, R * K, axis=1)                     # [B, 96]
    alphh[:, :NR, :] = rep.reshape(NCORES, S, NR).transpose(0, 2, 1)

    return xpad, wth, a3h, b3h, alphh


def kernel(x, alphas, W, lora_A, lora_B):
    from concourse.bass_utils import run_bass_kernel_spmd

    if "nc" not in _CACHE:
        _CACHE["nc"] = _build_nc()
    nc = _CACHE["nc"]

    xpad, wth, a3h, b3h, alphh = _host_prep(x, alphas, W, lora_A, lora_B)
    in_maps = [
        {"xp": np.ascontiguousarray(xpad[c]), "wt": wth, "a3": a3h, "b3": b3h,
         "alph": np.ascontiguousarray(alphh[c])}
        for c in range(NCORES)
    ]
    res = run_bass_kernel_spmd(nc, in_maps, list(range(NCORES)))
    out = np.empty((B, COUT, H, W_SP), np.float32)
    for c in range(NCORES):
        out[c * S:(c + 1) * S] = res.results[c]["out"].reshape(S, COUT, H, W_SP)
    return out

